# revision 13
# baseline (speedup 1.0000x reference)
"""Trainium2 Bass kernel for nn_LinearTextEmbedding_57604101374655.

Reference computation:
    out[c, x, y] = 1.0 if |bits[(x*1024 + y) % 4096]| > 0.5 else 0.0
    out shape (48, 1024, 1024) f32; all 48 channels identical; rows repeat
    with period 4 (4096 = 4*1024), so each channel is a 4x1024 pattern
    tiled 256x vertically.

Sharding: channel-parallel over 8 cores, 6 channels per core (bits
replicated). Each core writes its 24 MiB slab; host concatenates.

Per-core kernel (memory-regime: ~24 MiB HBM writes dominate, roofline
~70 us at ~358 GB/s HBM-write per core). Raw Bass blocks (no Tile - the
kernel is 8 instructions and Tile's tail barrier costs ~10 us):
  1. one DMA loads bits into a [128,1024] SBUF tile via a 0-stride
     broadcast source (partition p gets pattern row p%4),
  2. one DVE tensor_scalar thresholds the tile (|x|>0.5 -> 1/0),
  3. channel DMAs write 4 MiB each, source AP repeating the tile 8x
     via a 0-stride dim.
"""

import os
import sys

import numpy as np

sys.path.insert(0, "/opt/trn_rl_repo")

import concourse.bass as bass
from concourse import mybir
from concourse.bass_utils import run_bass_kernel_spmd

N_BITS = 4096
CHANNELS = 48
WIDTH = 1024
HEIGHT = 1024
N_CORES = 8
CH_PER_CORE = CHANNELS // N_CORES  # 6
CH_ELEMS = WIDTH * HEIGHT  # 1048576 elements per channel
P = 128

# Variants: "a" = 32 loads + 6 channel DMAs on SP; "a2" = same loads,
# channel DMAs alternate SP/ACT rings; "mega" = single 24 MiB DMA;
# "c" = 8 dual-ring loads into 32 partitions + DVE quarter-copies +
# dual-ring channel DMAs (fastest prologue).
VARIANT = os.environ.get("BASS_KERNEL_VARIANT", "c")

_CACHE = {}


def _build_nc(variant: str) -> bass.Bass:
    nc = bass.Bass()
    f32 = mybir.dt.float32
    bits = nc.declare_dram_parameter("bits", [N_BITS], f32, isOutput=False)
    out = nc.declare_dram_parameter(
        "out", [CH_PER_CORE * CH_ELEMS], f32, isOutput=True
    )

    with (
        nc.Block() as block,
        nc.semaphore("dma_sem") as dma_sem,
        nc.semaphore("v_sem") as v_sem,
        nc.sbuf_tensor("t_load", [P, 1024], f32) as t_load,
        nc.sbuf_tensor("t_pat", [P, 1024], f32) as t_pat,
    ):
        # Load bits so partition p holds bits[(p%4)*1024 : +1024]: 32
        # independent plain DMAs re-reading the 16 KiB vector (stride-0
        # broadcast sources are miscompiled for multi-partition dests —
        # verified on HW — so replicate by issuing one load per 4-row
        # group; they overlap in flight).
        load_src = bits[:].rearrange("(b w) -> b w", b=4)
        # channel DMA source: re-read the 512 KiB pattern tile 8x
        src_big = t_pat[:, :].unsqueeze(1).broadcast_to((P, 8, 1024))

        def chan_dst(c):
            # channel rows r = j*128 + p; r % 4 == p % 4 since 128 % 4 == 0
            return out[c * CH_ELEMS : (c + 1) * CH_ELEMS].rearrange(
                "(j p w) -> p j w", j=8, p=P
            )

        if variant.startswith("c"):
            # 8 loads fill partitions 0..31, split across both HWDGE
            # rings to halve the per-instruction issue serialization.
            @block.sync
            def _(sync):
                for a in range(4):
                    sync.dma_start(
                        out=t_load[4 * a : 4 * a + 4, :], in_=load_src
                    ).then_inc(dma_sem, 16)

            @block.scalar
            def _(scalar):
                for a in range(4, 8):
                    scalar.dma_start(
                        out=t_load[4 * a : 4 * a + 4, :], in_=load_src
                    ).then_inc(dma_sem, 16)

            @block.vector
            def _(vector):
                vector.wait_ge(dma_sem, 16 * 8)
                # threshold: |x| > 0.5  <=>  x*x > 0.25 (exact in f32)
                vector.tensor_mul(
                    t_pat[0:32, :], t_load[0:32, :], t_load[0:32, :]
                )
                vector.tensor_scalar(
                    out=t_pat[0:32, :],
                    in0=t_pat[0:32, :],
                    scalar1=0.25,
                    scalar2=None,
                    op0=mybir.AluOpType.is_gt,
                )
                # replicate to all partitions; compute-engine partition
                # bases must be quarter-aligned (0/32/64/96), which these
                # are.
                vector.tensor_copy(t_pat[32:64, :], t_pat[0:32, :])
                vector.tensor_copy(t_pat[64:128, :], t_pat[0:64, :]).then_inc(
                    v_sem, 1
                )

            if variant == "c":

                @block.sync
                def _(sync):
                    sync.wait_ge(v_sem, 1)
                    for c in range(0, CH_PER_CORE, 2):
                        sync.dma_start(
                            out=chan_dst(c), in_=src_big
                        ).then_inc(dma_sem, 16)
                    sync.wait_ge(dma_sem, 16 * (8 + CH_PER_CORE))

                @block.scalar
                def _(scalar):
                    scalar.wait_ge(v_sem, 1)
                    for c in range(1, CH_PER_CORE, 2):
                        scalar.dma_start(
                            out=chan_dst(c), in_=src_big
                        ).then_inc(dma_sem, 16)

            elif variant == "c_mega":
                # single 24 MiB write: no instruction boundaries, the 16
                # SDMA engines stream 6144 descriptors continuously
                mega_src = t_pat[:, :].unsqueeze(1).broadcast_to(
                    (P, 8 * CH_PER_CORE, 1024)
                )
                mega_dst = out[:].rearrange(
                    "(j p w) -> p j w", j=8 * CH_PER_CORE, p=P
                )

                @block.sync
                def _(sync):
                    sync.wait_ge(v_sem, 1)
                    sync.dma_start(out=mega_dst, in_=mega_src).then_inc(
                        dma_sem, 16
                    )
                    sync.wait_ge(dma_sem, 16 * 9)

            elif variant == "c_mega2":
                # two 12 MiB writes, one per HWDGE ring
                half = 4 * CH_PER_CORE  # j-extent of half the slab
                h_src = t_pat[:, :].unsqueeze(1).broadcast_to(
                    (P, half, 1024)
                )

                def half_dst(h):
                    n = CH_PER_CORE * CH_ELEMS // 2
                    return out[h * n : (h + 1) * n].rearrange(
                        "(j p w) -> p j w", j=half, p=P
                    )

                @block.sync
                def _(sync):
                    sync.wait_ge(v_sem, 1)
                    sync.dma_start(out=half_dst(0), in_=h_src).then_inc(
                        dma_sem, 16
                    )
                    sync.wait_ge(dma_sem, 16 * 10)

                @block.scalar
                def _(scalar):
                    scalar.wait_ge(v_sem, 1)
                    scalar.dma_start(out=half_dst(1), in_=h_src).then_inc(
                        dma_sem, 16
                    )

            else:
                raise ValueError(f"unknown variant {variant!r}")
            return nc

        @block.sync
        def _(sync):
            for a in range(32):
                sync.dma_start(
                    out=t_load[4 * a : 4 * a + 4, :], in_=load_src
                ).then_inc(dma_sem, 16)

        @block.vector
        def _(vector):
            vector.wait_ge(dma_sem, 16 * 32)
            # threshold: |x| > 0.5  <=>  x*x > 0.25 (exact in f32)
            vector.tensor_mul(t_pat[:, :], t_load[:, :], t_load[:, :])
            vector.tensor_scalar(
                out=t_pat[:, :],
                in0=t_pat[:, :],
                scalar1=0.25,
                scalar2=None,
                op0=mybir.AluOpType.is_gt,
            ).then_inc(v_sem, 1)

        if variant == "a":

            @block.sync
            def _(sync):
                sync.wait_ge(v_sem, 1)
                for c in range(CH_PER_CORE):
                    sync.dma_start(out=chan_dst(c), in_=src_big).then_inc(
                        dma_sem, 16
                    )
                sync.wait_ge(dma_sem, 16 * (32 + CH_PER_CORE))

        elif variant == "a2":

            @block.scalar
            def _(scalar):
                scalar.wait_ge(v_sem, 1)
                for c in range(0, CH_PER_CORE, 2):
                    scalar.dma_start(out=chan_dst(c), in_=src_big).then_inc(
                        dma_sem, 16
                    )

            @block.sync
            def _(sync):
                sync.wait_ge(v_sem, 1)
                for c in range(1, CH_PER_CORE, 2):
                    sync.dma_start(out=chan_dst(c), in_=src_big).then_inc(
                        dma_sem, 16
                    )
                sync.wait_ge(dma_sem, 16 * (32 + CH_PER_CORE))

        elif variant == "mega":
            mega_src = t_pat[:, :].unsqueeze(1).broadcast_to(
                (P, 8 * CH_PER_CORE, 1024)
            )
            mega_dst = out[:].rearrange(
                "(j p w) -> p j w", j=8 * CH_PER_CORE, p=P
            )

            @block.sync
            def _(sync):
                sync.wait_ge(v_sem, 1)
                sync.dma_start(out=mega_dst, in_=mega_src).then_inc(
                    dma_sem, 16
                )
                sync.wait_ge(dma_sem, 16 * 33)

        else:
            raise ValueError(f"unknown variant {variant!r}")
    return nc


def _get_nc(variant: str) -> bass.Bass:
    if variant not in _CACHE:
        _CACHE[variant] = _build_nc(variant)
    return _CACHE[variant]


def kernel(bits: np.ndarray, **_kw) -> np.ndarray:
    bits = np.ascontiguousarray(bits, dtype=np.float32)
    nc = _get_nc(VARIANT)
    core_ids = list(range(N_CORES))
    in_maps = [{"bits": bits} for _ in core_ids]
    res = run_bass_kernel_spmd(nc, in_maps, core_ids)
    slabs = [
        res.results[i]["out"].reshape(CH_PER_CORE, WIDTH, HEIGHT)
        for i in range(N_CORES)
    ]
    return np.concatenate(slabs, axis=0)


# revision 14
# speedup vs baseline: 1.0505x; 1.0505x over previous
"""Trainium2 Bass kernel for nn_LinearTextEmbedding_57604101374655.

Reference computation:
    out[c, x, y] = 1.0 if |bits[(x*1024 + y) % 4096]| > 0.5 else 0.0
    out shape (48, 1024, 1024) f32; all 48 channels identical; rows repeat
    with period 4 (4096 = 4*1024), so each channel is a 4x1024 pattern
    tiled 256x vertically.

Sharding: channel-parallel over 8 cores, 6 channels per core (bits
replicated). Each core writes its 24 MiB slab; host concatenates.

Per-core kernel (memory-regime: ~24 MiB HBM writes dominate, roofline
~70 us at ~358 GB/s HBM-write per core). Raw Bass blocks (no Tile - the
kernel is 8 instructions and Tile's tail barrier costs ~10 us):
  1. one DMA loads bits into a [128,1024] SBUF tile via a 0-stride
     broadcast source (partition p gets pattern row p%4),
  2. one DVE tensor_scalar thresholds the tile (|x|>0.5 -> 1/0),
  3. channel DMAs write 4 MiB each, source AP repeating the tile 8x
     via a 0-stride dim.
"""

import os
import sys

import numpy as np

sys.path.insert(0, "/opt/trn_rl_repo")

import concourse.bass as bass
from concourse import mybir
from concourse.bass_utils import run_bass_kernel_spmd

N_BITS = 4096
CHANNELS = 48
WIDTH = 1024
HEIGHT = 1024
N_CORES = 8
CH_PER_CORE = CHANNELS // N_CORES  # 6
CH_ELEMS = WIDTH * HEIGHT  # 1048576 elements per channel
P = 128

# Variants: "a" = 32 loads + 6 channel DMAs on SP; "a2" = same loads,
# channel DMAs alternate SP/ACT rings; "mega" = single 24 MiB DMA;
# "c" = 8 dual-ring loads into 32 partitions + DVE quarter-copies +
# dual-ring channel DMAs (fastest prologue).
VARIANT = os.environ.get("BASS_KERNEL_VARIANT", "c")

_CACHE = {}


def _build_nc(variant: str) -> bass.Bass:
    nc = bass.Bass()
    f32 = mybir.dt.float32
    bits = nc.declare_dram_parameter("bits", [N_BITS], f32, isOutput=False)
    out = nc.declare_dram_parameter(
        "out", [CH_PER_CORE * CH_ELEMS], f32, isOutput=True
    )

    with (
        nc.Block() as block,
        nc.semaphore("dma_sem") as dma_sem,
        nc.semaphore("v_sem") as v_sem,
        nc.sbuf_tensor("t_load", [P, 1024], f32) as t_load,
        nc.sbuf_tensor("t_pat", [P, 1024], f32) as t_pat,
    ):
        # Load bits so partition p holds bits[(p%4)*1024 : +1024]: 32
        # independent plain DMAs re-reading the 16 KiB vector (stride-0
        # broadcast sources are miscompiled for multi-partition dests —
        # verified on HW — so replicate by issuing one load per 4-row
        # group; they overlap in flight).
        load_src = bits[:].rearrange("(b w) -> b w", b=4)
        # channel DMA source: re-read the 512 KiB pattern tile 8x
        src_big = t_pat[:, :].unsqueeze(1).broadcast_to((P, 8, 1024))

        def chan_dst(c):
            # channel rows r = j*128 + p; r % 4 == p % 4 since 128 % 4 == 0
            return out[c * CH_ELEMS : (c + 1) * CH_ELEMS].rearrange(
                "(j p w) -> p j w", j=8, p=P
            )

        if variant.startswith("c"):
            # 8 loads fill partitions 0..31, split across both HWDGE
            # rings to halve the per-instruction issue serialization.
            @block.sync
            def _(sync):
                for a in range(4):
                    sync.dma_start(
                        out=t_load[4 * a : 4 * a + 4, :], in_=load_src
                    ).then_inc(dma_sem, 16)

            @block.scalar
            def _(scalar):
                for a in range(4, 8):
                    scalar.dma_start(
                        out=t_load[4 * a : 4 * a + 4, :], in_=load_src
                    ).then_inc(dma_sem, 16)

            @block.vector
            def _(vector):
                vector.wait_ge(dma_sem, 16 * 8)
                # threshold: |x| > 0.5  <=>  x*x > 0.25 (exact in f32)
                vector.tensor_mul(
                    t_pat[0:32, :], t_load[0:32, :], t_load[0:32, :]
                )
                vector.tensor_scalar(
                    out=t_pat[0:32, :],
                    in0=t_pat[0:32, :],
                    scalar1=0.25,
                    scalar2=None,
                    op0=mybir.AluOpType.is_gt,
                )
                # replicate to all partitions; compute-engine partition
                # bases must be quarter-aligned (0/32/64/96), which these
                # are.
                vector.tensor_copy(t_pat[32:64, :], t_pat[0:32, :])
                vector.tensor_copy(t_pat[64:128, :], t_pat[0:64, :]).then_inc(
                    v_sem, 1
                )

            if variant == "c" or variant.startswith("c_s"):
                # writes split into 6*S instructions of (4/S) MiB,
                # alternating across the two HWDGE rings
                S = 1 if variant == "c" else int(variant[3:])
                jj = 8 // S  # j-extent per write
                n_wr = CH_PER_CORE * S
                src_s = t_pat[:, :].unsqueeze(1).broadcast_to(
                    (P, jj, 1024)
                )

                def wr_dst(i):
                    n = CH_ELEMS // S
                    return out[i * n : (i + 1) * n].rearrange(
                        "(j p w) -> p j w", j=jj, p=P
                    )

                @block.sync
                def _(sync):
                    sync.wait_ge(v_sem, 1)
                    for i in range(0, n_wr, 2):
                        sync.dma_start(out=wr_dst(i), in_=src_s).then_inc(
                            dma_sem, 16
                        )
                    sync.wait_ge(dma_sem, 16 * (8 + n_wr))

                @block.scalar
                def _(scalar):
                    scalar.wait_ge(v_sem, 1)
                    for i in range(1, n_wr, 2):
                        scalar.dma_start(out=wr_dst(i), in_=src_s).then_inc(
                            dma_sem, 16
                        )

            elif variant == "c_mega":
                # single 24 MiB write: no instruction boundaries, the 16
                # SDMA engines stream 6144 descriptors continuously
                mega_src = t_pat[:, :].unsqueeze(1).broadcast_to(
                    (P, 8 * CH_PER_CORE, 1024)
                )
                mega_dst = out[:].rearrange(
                    "(j p w) -> p j w", j=8 * CH_PER_CORE, p=P
                )

                @block.sync
                def _(sync):
                    sync.wait_ge(v_sem, 1)
                    sync.dma_start(out=mega_dst, in_=mega_src).then_inc(
                        dma_sem, 16
                    )
                    sync.wait_ge(dma_sem, 16 * 9)

            elif variant == "c_mega2":
                # two 12 MiB writes, one per HWDGE ring
                half = 4 * CH_PER_CORE  # j-extent of half the slab
                h_src = t_pat[:, :].unsqueeze(1).broadcast_to(
                    (P, half, 1024)
                )

                def half_dst(h):
                    n = CH_PER_CORE * CH_ELEMS // 2
                    return out[h * n : (h + 1) * n].rearrange(
                        "(j p w) -> p j w", j=half, p=P
                    )

                @block.sync
                def _(sync):
                    sync.wait_ge(v_sem, 1)
                    sync.dma_start(out=half_dst(0), in_=h_src).then_inc(
                        dma_sem, 16
                    )
                    sync.wait_ge(dma_sem, 16 * 10)

                @block.scalar
                def _(scalar):
                    scalar.wait_ge(v_sem, 1)
                    scalar.dma_start(out=half_dst(1), in_=h_src).then_inc(
                        dma_sem, 16
                    )

            else:
                raise ValueError(f"unknown variant {variant!r}")
            return nc

        @block.sync
        def _(sync):
            for a in range(32):
                sync.dma_start(
                    out=t_load[4 * a : 4 * a + 4, :], in_=load_src
                ).then_inc(dma_sem, 16)

        @block.vector
        def _(vector):
            vector.wait_ge(dma_sem, 16 * 32)
            # threshold: |x| > 0.5  <=>  x*x > 0.25 (exact in f32)
            vector.tensor_mul(t_pat[:, :], t_load[:, :], t_load[:, :])
            vector.tensor_scalar(
                out=t_pat[:, :],
                in0=t_pat[:, :],
                scalar1=0.25,
                scalar2=None,
                op0=mybir.AluOpType.is_gt,
            ).then_inc(v_sem, 1)

        if variant == "a":

            @block.sync
            def _(sync):
                sync.wait_ge(v_sem, 1)
                for c in range(CH_PER_CORE):
                    sync.dma_start(out=chan_dst(c), in_=src_big).then_inc(
                        dma_sem, 16
                    )
                sync.wait_ge(dma_sem, 16 * (32 + CH_PER_CORE))

        elif variant == "a2":

            @block.scalar
            def _(scalar):
                scalar.wait_ge(v_sem, 1)
                for c in range(0, CH_PER_CORE, 2):
                    scalar.dma_start(out=chan_dst(c), in_=src_big).then_inc(
                        dma_sem, 16
                    )

            @block.sync
            def _(sync):
                sync.wait_ge(v_sem, 1)
                for c in range(1, CH_PER_CORE, 2):
                    sync.dma_start(out=chan_dst(c), in_=src_big).then_inc(
                        dma_sem, 16
                    )
                sync.wait_ge(dma_sem, 16 * (32 + CH_PER_CORE))

        elif variant == "mega":
            mega_src = t_pat[:, :].unsqueeze(1).broadcast_to(
                (P, 8 * CH_PER_CORE, 1024)
            )
            mega_dst = out[:].rearrange(
                "(j p w) -> p j w", j=8 * CH_PER_CORE, p=P
            )

            @block.sync
            def _(sync):
                sync.wait_ge(v_sem, 1)
                sync.dma_start(out=mega_dst, in_=mega_src).then_inc(
                    dma_sem, 16
                )
                sync.wait_ge(dma_sem, 16 * 33)

        else:
            raise ValueError(f"unknown variant {variant!r}")
    return nc


def _get_nc(variant: str) -> bass.Bass:
    if variant not in _CACHE:
        _CACHE[variant] = _build_nc(variant)
    return _CACHE[variant]


def kernel(bits: np.ndarray, **_kw) -> np.ndarray:
    bits = np.ascontiguousarray(bits, dtype=np.float32)
    nc = _get_nc(VARIANT)
    core_ids = list(range(N_CORES))
    in_maps = [{"bits": bits} for _ in core_ids]
    res = run_bass_kernel_spmd(nc, in_maps, core_ids)
    slabs = [
        res.results[i]["out"].reshape(CH_PER_CORE, WIDTH, HEIGHT)
        for i in range(N_CORES)
    ]
    return np.concatenate(slabs, axis=0)


# revision 16
# speedup vs baseline: 1.2031x; 1.1453x over previous
"""Trainium2 Bass kernel for nn_LinearTextEmbedding_57604101374655.

Reference computation:
    out[c, x, y] = 1.0 if |bits[(x*1024 + y) % 4096]| > 0.5 else 0.0
    out shape (48, 1024, 1024) f32; all 48 channels identical; rows repeat
    with period 4 (4096 = 4*1024), so each channel is a 4x1024 pattern
    tiled 256x vertically.

Sharding: channel-parallel over 8 cores, 6 channels per core (bits
replicated). Each core writes its 24 MiB slab; host concatenates.

Per-core kernel (memory-regime: ~24 MiB HBM writes dominate, roofline
~70 us at ~358 GB/s HBM-write per core). Raw Bass blocks (no Tile - the
kernel is 8 instructions and Tile's tail barrier costs ~10 us):
  1. one DMA loads bits into a [128,1024] SBUF tile via a 0-stride
     broadcast source (partition p gets pattern row p%4),
  2. one DVE tensor_scalar thresholds the tile (|x|>0.5 -> 1/0),
  3. channel DMAs write 4 MiB each, source AP repeating the tile 8x
     via a 0-stride dim.
"""

import os
import sys

import numpy as np

sys.path.insert(0, "/opt/trn_rl_repo")

import concourse.bass as bass
from concourse import mybir
from concourse.bass_utils import run_bass_kernel_spmd

N_BITS = 4096
CHANNELS = 48
WIDTH = 1024
HEIGHT = 1024
N_CORES = 8
CH_PER_CORE = CHANNELS // N_CORES  # 6
CH_ELEMS = WIDTH * HEIGHT  # 1048576 elements per channel
P = 128

# Variants: "a" = 32 loads + 6 channel DMAs on SP; "a2" = same loads,
# channel DMAs alternate SP/ACT rings; "mega" = single 24 MiB DMA;
# "c" = 8 dual-ring loads into 32 partitions + DVE quarter-copies +
# dual-ring channel DMAs (fastest prologue).
VARIANT = os.environ.get("BASS_KERNEL_VARIANT", "c")

_CACHE = {}


def _build_nc(variant: str) -> bass.Bass:
    nc = bass.Bass()
    f32 = mybir.dt.float32
    out = nc.declare_dram_parameter(
        "out", [CH_PER_CORE * CH_ELEMS], f32, isOutput=True
    )

    if variant.startswith("d"):
        # host passes bits tiled 32x (512 KiB): partition p gets pattern
        # row p%4 with a single plain load; threshold is 2 DVE ops; then
        # split writes across both HWDGE rings (2 MiB instructions).
        S = 2
        jj = 8 // S
        n_wr = CH_PER_CORE * S
        bits_rep = nc.declare_dram_parameter(
            "bits_rep", [32 * N_BITS], f32, isOutput=False
        )
        with (
            nc.Block() as block,
            nc.semaphore("dma_sem") as dma_sem,
            nc.semaphore("v_sem") as v_sem,
            nc.sbuf_tensor("t_load", [P, 1024], f32) as t_load,
            nc.sbuf_tensor("t_pat", [P, 1024], f32) as t_pat,
        ):
            src_s = t_pat[:, :].unsqueeze(1).broadcast_to((P, jj, 1024))

            def wr_dst(i):
                n = CH_ELEMS // S
                return out[i * n : (i + 1) * n].rearrange(
                    "(j p w) -> p j w", j=jj, p=P
                )

            @block.sync
            def _(sync):
                sync.dma_start(
                    out=t_load[:, :],
                    in_=bits_rep[:].rearrange("(p w) -> p w", p=P),
                ).then_inc(dma_sem, 16)

            @block.vector
            def _(vector):
                vector.wait_ge(dma_sem, 16)
                # threshold: |x| > 0.5  <=>  x*x > 0.25 (exact in f32)
                vector.tensor_mul(t_pat[:, :], t_load[:, :], t_load[:, :])
                vector.tensor_scalar(
                    out=t_pat[:, :],
                    in0=t_pat[:, :],
                    scalar1=0.25,
                    scalar2=None,
                    op0=mybir.AluOpType.is_gt,
                ).then_inc(v_sem, 1)

            @block.sync
            def _(sync):
                sync.wait_ge(v_sem, 1)
                for i in range(0, n_wr, 2):
                    sync.dma_start(out=wr_dst(i), in_=src_s).then_inc(
                        dma_sem, 16
                    )
                sync.wait_ge(dma_sem, 16 * (1 + n_wr))

            @block.scalar
            def _(scalar):
                scalar.wait_ge(v_sem, 1)
                for i in range(1, n_wr, 2):
                    scalar.dma_start(out=wr_dst(i), in_=src_s).then_inc(
                        dma_sem, 16
                    )

        return nc

    bits = nc.declare_dram_parameter("bits", [N_BITS], f32, isOutput=False)

    with (
        nc.Block() as block,
        nc.semaphore("dma_sem") as dma_sem,
        nc.semaphore("v_sem") as v_sem,
        nc.sbuf_tensor("t_load", [P, 1024], f32) as t_load,
        nc.sbuf_tensor("t_pat", [P, 1024], f32) as t_pat,
    ):
        # Load bits so partition p holds bits[(p%4)*1024 : +1024]: 32
        # independent plain DMAs re-reading the 16 KiB vector (stride-0
        # broadcast sources are miscompiled for multi-partition dests —
        # verified on HW — so replicate by issuing one load per 4-row
        # group; they overlap in flight).
        load_src = bits[:].rearrange("(b w) -> b w", b=4)
        # channel DMA source: re-read the 512 KiB pattern tile 8x
        src_big = t_pat[:, :].unsqueeze(1).broadcast_to((P, 8, 1024))

        def chan_dst(c):
            # channel rows r = j*128 + p; r % 4 == p % 4 since 128 % 4 == 0
            return out[c * CH_ELEMS : (c + 1) * CH_ELEMS].rearrange(
                "(j p w) -> p j w", j=8, p=P
            )

        if variant.startswith("c"):
            # 8 loads fill partitions 0..31, split across both HWDGE
            # rings to halve the per-instruction issue serialization.
            @block.sync
            def _(sync):
                for a in range(4):
                    sync.dma_start(
                        out=t_load[4 * a : 4 * a + 4, :], in_=load_src
                    ).then_inc(dma_sem, 16)

            @block.scalar
            def _(scalar):
                for a in range(4, 8):
                    scalar.dma_start(
                        out=t_load[4 * a : 4 * a + 4, :], in_=load_src
                    ).then_inc(dma_sem, 16)

            @block.vector
            def _(vector):
                vector.wait_ge(dma_sem, 16 * 8)
                # threshold: |x| > 0.5  <=>  x*x > 0.25 (exact in f32)
                vector.tensor_mul(
                    t_pat[0:32, :], t_load[0:32, :], t_load[0:32, :]
                )
                vector.tensor_scalar(
                    out=t_pat[0:32, :],
                    in0=t_pat[0:32, :],
                    scalar1=0.25,
                    scalar2=None,
                    op0=mybir.AluOpType.is_gt,
                )
                # replicate to all partitions; compute-engine partition
                # bases must be quarter-aligned (0/32/64/96), which these
                # are.
                vector.tensor_copy(t_pat[32:64, :], t_pat[0:32, :])
                vector.tensor_copy(t_pat[64:128, :], t_pat[0:64, :]).then_inc(
                    v_sem, 1
                )

            if variant == "c" or variant.startswith("c_s"):
                # writes split into 6*S instructions of (4/S) MiB,
                # alternating across the two HWDGE rings
                S = 1 if variant == "c" else int(variant[3:])
                jj = 8 // S  # j-extent per write
                n_wr = CH_PER_CORE * S
                src_s = t_pat[:, :].unsqueeze(1).broadcast_to(
                    (P, jj, 1024)
                )

                def wr_dst(i):
                    n = CH_ELEMS // S
                    return out[i * n : (i + 1) * n].rearrange(
                        "(j p w) -> p j w", j=jj, p=P
                    )

                @block.sync
                def _(sync):
                    sync.wait_ge(v_sem, 1)
                    for i in range(0, n_wr, 2):
                        sync.dma_start(out=wr_dst(i), in_=src_s).then_inc(
                            dma_sem, 16
                        )
                    sync.wait_ge(dma_sem, 16 * (8 + n_wr))

                @block.scalar
                def _(scalar):
                    scalar.wait_ge(v_sem, 1)
                    for i in range(1, n_wr, 2):
                        scalar.dma_start(out=wr_dst(i), in_=src_s).then_inc(
                            dma_sem, 16
                        )

            elif variant == "c_mega":
                # single 24 MiB write: no instruction boundaries, the 16
                # SDMA engines stream 6144 descriptors continuously
                mega_src = t_pat[:, :].unsqueeze(1).broadcast_to(
                    (P, 8 * CH_PER_CORE, 1024)
                )
                mega_dst = out[:].rearrange(
                    "(j p w) -> p j w", j=8 * CH_PER_CORE, p=P
                )

                @block.sync
                def _(sync):
                    sync.wait_ge(v_sem, 1)
                    sync.dma_start(out=mega_dst, in_=mega_src).then_inc(
                        dma_sem, 16
                    )
                    sync.wait_ge(dma_sem, 16 * 9)

            elif variant == "c_mega2":
                # two 12 MiB writes, one per HWDGE ring
                half = 4 * CH_PER_CORE  # j-extent of half the slab
                h_src = t_pat[:, :].unsqueeze(1).broadcast_to(
                    (P, half, 1024)
                )

                def half_dst(h):
                    n = CH_PER_CORE * CH_ELEMS // 2
                    return out[h * n : (h + 1) * n].rearrange(
                        "(j p w) -> p j w", j=half, p=P
                    )

                @block.sync
                def _(sync):
                    sync.wait_ge(v_sem, 1)
                    sync.dma_start(out=half_dst(0), in_=h_src).then_inc(
                        dma_sem, 16
                    )
                    sync.wait_ge(dma_sem, 16 * 10)

                @block.scalar
                def _(scalar):
                    scalar.wait_ge(v_sem, 1)
                    scalar.dma_start(out=half_dst(1), in_=h_src).then_inc(
                        dma_sem, 16
                    )

            else:
                raise ValueError(f"unknown variant {variant!r}")
            return nc

        @block.sync
        def _(sync):
            for a in range(32):
                sync.dma_start(
                    out=t_load[4 * a : 4 * a + 4, :], in_=load_src
                ).then_inc(dma_sem, 16)

        @block.vector
        def _(vector):
            vector.wait_ge(dma_sem, 16 * 32)
            # threshold: |x| > 0.5  <=>  x*x > 0.25 (exact in f32)
            vector.tensor_mul(t_pat[:, :], t_load[:, :], t_load[:, :])
            vector.tensor_scalar(
                out=t_pat[:, :],
                in0=t_pat[:, :],
                scalar1=0.25,
                scalar2=None,
                op0=mybir.AluOpType.is_gt,
            ).then_inc(v_sem, 1)

        if variant == "a":

            @block.sync
            def _(sync):
                sync.wait_ge(v_sem, 1)
                for c in range(CH_PER_CORE):
                    sync.dma_start(out=chan_dst(c), in_=src_big).then_inc(
                        dma_sem, 16
                    )
                sync.wait_ge(dma_sem, 16 * (32 + CH_PER_CORE))

        elif variant == "a2":

            @block.scalar
            def _(scalar):
                scalar.wait_ge(v_sem, 1)
                for c in range(0, CH_PER_CORE, 2):
                    scalar.dma_start(out=chan_dst(c), in_=src_big).then_inc(
                        dma_sem, 16
                    )

            @block.sync
            def _(sync):
                sync.wait_ge(v_sem, 1)
                for c in range(1, CH_PER_CORE, 2):
                    sync.dma_start(out=chan_dst(c), in_=src_big).then_inc(
                        dma_sem, 16
                    )
                sync.wait_ge(dma_sem, 16 * (32 + CH_PER_CORE))

        elif variant == "mega":
            mega_src = t_pat[:, :].unsqueeze(1).broadcast_to(
                (P, 8 * CH_PER_CORE, 1024)
            )
            mega_dst = out[:].rearrange(
                "(j p w) -> p j w", j=8 * CH_PER_CORE, p=P
            )

            @block.sync
            def _(sync):
                sync.wait_ge(v_sem, 1)
                sync.dma_start(out=mega_dst, in_=mega_src).then_inc(
                    dma_sem, 16
                )
                sync.wait_ge(dma_sem, 16 * 33)

        else:
            raise ValueError(f"unknown variant {variant!r}")
    return nc


def _get_nc(variant: str) -> bass.Bass:
    if variant not in _CACHE:
        _CACHE[variant] = _build_nc(variant)
    return _CACHE[variant]


def kernel(bits: np.ndarray, **_kw) -> np.ndarray:
    bits = np.ascontiguousarray(bits, dtype=np.float32)
    nc = _get_nc(VARIANT)
    core_ids = list(range(N_CORES))
    if VARIANT.startswith("d"):
        rep = np.tile(bits, 32)
        in_maps = [{"bits_rep": rep} for _ in core_ids]
    else:
        in_maps = [{"bits": bits} for _ in core_ids]
    res = run_bass_kernel_spmd(nc, in_maps, core_ids)
    slabs = [
        res.results[i]["out"].reshape(CH_PER_CORE, WIDTH, HEIGHT)
        for i in range(N_CORES)
    ]
    return np.concatenate(slabs, axis=0)
